# revision 42
# baseline (speedup 1.0000x reference)
"""Trainium2 Bass kernel for ExtensibleAttention (sparse_attention).

Strategy: data-parallel over the 65536 tokens (N*L flattened) across 8
NeuronCores; the small 256-dim projection weights are replicated. All
per-token math is fused into one pass per 512-token tile:

  q/k/v/pos projections as PE matmuls in [C, T] layout (channel on
  partitions, token on free dim). pos@Wpos is computed ONCE per tile and
  added into the q/k projections on the vector engine (saves 4 of 44
  matmul streams per tile vs accumulating Wpos into both q and k on the
  PE). Offset MLP (relu + second projection), the grid-sample weight w,
  softmax over K=4 sample points, and the final out-projection all
  on-chip.

Inputs are pre-transposed to [C, T] on the host (numpy) so the kernel
needs no on-chip transposes. Head reductions, the k-broadcast of qk, the
sum over K, and the head->channel broadcast of wv are matmuls against
small constant 0/1 matrices. Wo2 columns are host-permuted to (c,h,k)
order so the grid-sample weight product is a single vector multiply.
The out-projection runs channel-major ([C, T] output, transposed back
to token-major on the host).

Scheduling: a skewed software pipeline emits, per iteration i,
  qk(i-1) / outproj(i-2) first (inputs long ready, keeps PE dense),
  then stage1(i) interleaved with the softmax chain of tile i-1 so that
  ACT (t1/t2/e) and DVE (w/lg) run a full iteration ahead of the PE
  matmuls (s1/s2/wvx) that consume them. Per-engine program order =
  emission order, so the interleave directly controls each queue.
"""

import numpy as np
from contextlib import ExitStack

import concourse.bacc as bacc
import concourse.tile as tile
from concourse import mybir

F32 = mybir.dt.float32
F32R = mybir.dt.float32r
BF16 = mybir.dt.bfloat16
AF = mybir.ActivationFunctionType

N, L, C, H, KP, D = 4, 16384, 256, 8, 4, 32
NCORES = 8
TOKS = N * L // NCORES  # 8192 tokens per core
TLOAD = 512             # tokens per DMA load tile
SIGMA = float(1.0 / np.sqrt(D))


def _build(toks=TOKS, tload=TLOAD, with_bias=False):
    nc = bacc.Bacc(trn_type="TRN2")
    dram = {}

    def din(name, shape, dt=None):
        dram[name] = nc.dram_tensor(name, list(shape), dt or F32R,
                                    kind="ExternalInput")
        return dram[name]

    xq = din("xq", (128, 2, toks))
    xk = din("xk", (128, 2, toks))
    xv = din("xv", (128, 2, toks))
    xp = din("xp", (128, 2, toks))
    ref = din("ref", (2, toks))
    for nm, shp in (("wq", (128, 2, 256)), ("wk", (128, 2, 256)),
                    ("wv", (128, 2, 256)), ("wp", (128, 2, 256)),
                    ("wo1", (128, 2, 512)), ("wo2", (128, 4, 64)),
                    ("wo", (128, 2, 256)), ("bo1", (128, 4)),
                    ("bwof", (64, 1)), ("smat", (64, 32)),
                    ("amat", (128, 64)), ("cmat", (32, 8)),
                    ("bmat", (8, 256)), ("pmat", (2, 64))):
        din(nm, shp)
    if with_bias:
        for nm in ("bqr", "bkr", "bpr", "bvr", "bor"):
            din(nm, (1, 256))
        din("ones", (1, 512))
    # channel-major bf16 output [p, chunk, t]; host upconverts and
    # transposes back to [t, 256] f32 (quantization adds ~4e-3 rel err,
    # well under the 2e-2 gate, and halves the output DMA)
    out = nc.dram_tensor("out", [128, 2, toks], BF16, kind="ExternalOutput")

    nload = toks // tload
    T = tload

    with tile.TileContext(nc) as tc, ExitStack() as ctx:
        singles = ctx.enter_context(tc.tile_pool(name="singles", bufs=1))
        inp = ctx.enter_context(tc.tile_pool(name="inp", bufs=4))
        work = ctx.enter_context(tc.tile_pool(name="work", bufs=2))
        psA = ctx.enter_context(tc.tile_pool(name="psA", bufs=3, space="PSUM"))
        psB = ctx.enter_context(tc.tile_pool(name="psB", bufs=5, space="PSUM"))

        def load1(name, shape, dt=F32R):
            t = singles.tile(list(shape), dt, name=f"sb_{name}")
            nc.sync.dma_start(out=t, in_=dram[name][:])
            return t

        mm = nc.tensor.matmul

        def load_tile(lt):
            t0 = lt * tload
            xp_t = inp.tile([128, 2, tload], F32R, tag="xp")
            nc.sync.dma_start(out=xp_t, in_=xp[:, :, t0:t0 + tload])
            xq_t = inp.tile([128, 2, tload], F32R, tag="xq")
            nc.sync.dma_start(out=xq_t, in_=xq[:, :, t0:t0 + tload])
            xk_t = inp.tile([128, 2, tload], F32R, tag="xk")
            nc.sync.dma_start(out=xk_t, in_=xk[:, :, t0:t0 + tload])
            xv_t = inp.tile([128, 2, tload], F32R, tag="xv")
            nc.sync.dma_start(out=xv_t, in_=xv[:, :, t0:t0 + tload])
            ref_t = inp.tile([2, tload], F32R, tag="ref")
            nc.sync.dma_start(out=ref_t, in_=ref[:, t0:t0 + tload])
            return xq_t, xp_t, xk_t, xv_t, ref_t

        # ---------- per-tile emit pieces (st = per-tile state dict) ----------

        def _proj(ps_tag, w_s, x_t, mc, b_s=None):
            m128 = slice(mc * 128, (mc + 1) * 128)
            ps = psA.tile([128, T], F32, tag="bigA", name=ps_tag)
            mm(ps, w_s[:, 0, m128], x_t[:, 0, :], start=True, stop=False)
            mm(ps, w_s[:, 1, m128], x_t[:, 1, :], start=False,
               stop=not with_bias)
            if with_bias:
                mm(ps, b_s[:, m128], ones_s[:, :T], start=False, stop=True)
            return ps

        def emit_chunk_pq(st, mc):
            """pp and q matmuls for one chunk + pp ACT copy."""
            xq_t, xp_t = st["ld"][0], st["ld"][1]
            bpr = bpr_s if with_bias else None
            bqr = bqr_s if with_bias else None
            st[f"pp_ps{mc}"] = _proj("pp_ps", wp_s, xp_t, mc, bpr)
            st[f"q_ps{mc}"] = _proj("q_ps", wq_s, xq_t, mc, bqr)
            nc.scalar.copy(st["pp"][:, mc, :], st[f"pp_ps{mc}"])

        def emit_chunk_k(st, mc):
            xk_t = st["ld"][2]
            bkr = bkr_s if with_bias else None
            st[f"k_ps{mc}"] = _proj("k_ps", wk_s, xk_t, mc, bkr)

        def emit_chunk_v(st, mc):
            xv_t = st["ld"][3]
            bvr = bvr_s if with_bias else None
            v_ps = _proj("v_ps", wv_s, xv_t, mc, bvr)
            nc.scalar.copy(st["v"][:, mc, :], v_ps)

        def emit_adds(st, mc):
            """DVE adds of pp into q/k (also the PSUM->SBUF move)."""
            nc.vector.tensor_add(st["tq"][:, mc, :], st[f"q_ps{mc}"],
                                 st["pp"][:, mc, :])
            nc.vector.tensor_add(st["tk"][:, mc, :], st[f"k_ps{mc}"],
                                 st["pp"][:, mc, :])

        def emit_mmuls(st):
            for mc in range(2):
                nc.gpsimd.tensor_mul(st["m"][:, mc, :], st["tq"][:, mc, :],
                                     st["tk"][:, mc, :])

        def emit_hid(st):
            xq_t = st["ld"][0]
            hid = work.tile([128, 4, T], F32R, tag="hid", bufs=1)
            st["hid"] = hid
            for j in range(4):
                h_ps = psB.tile([128, T], F32, tag="small")
                j128 = slice(j * 128, (j + 1) * 128)
                mm(h_ps, wo1_s[:, 0, j128], xq_t[:, 0, :], start=True, stop=False)
                mm(h_ps, wo1_s[:, 1, j128], xq_t[:, 1, :], start=False, stop=True)
                nc.scalar.activation(hid[:, j, :], h_ps, AF.Relu,
                                     bias=bo1_s[:, j:j + 1], scale=1.0)

        def emit_off(st):
            ref_t = st["ld"][4]
            off_ps = psB.tile([64, T], F32, tag="small")
            st["off"] = off_ps
            for j in range(4):
                mm(off_ps, wo2_s[:, j, :], st["hid"][:, j, :],
                   start=(j == 0), stop=False)
            mm(off_ps, pmat_s, ref_t, start=False, stop=True)

        def emit_qk(st):
            qk_ps = psB.tile([32, T], F32, tag="small")
            st["qk"] = qk_ps
            mm(qk_ps, amat_s[:, 0:32], st["m"][:, 0, :], start=True, stop=False)
            mm(qk_ps, amat_s[:, 32:64], st["m"][:, 1, :], start=False, stop=True)

        def emit_t1(st):
            t1_sb = work.tile([64, T], F32, tag="t1")
            nc.scalar.activation(t1_sb, st["off"], AF.Abs, bias=bwof_s, scale=1.0)
            st["t1"] = t1_sb

        def emit_nt(st):
            # nt = min(t1 - 1, 0) = -relu(1 - t1); the sign cancels in the
            # w = nt_x * nt_y product, so GPSIMD replaces the second ACT op
            nt_sb = work.tile([64, T], F32R, tag="t2", bufs=3)
            nc.gpsimd.tensor_scalar(nt_sb, st["t1"], 1.0, 0.0,
                                    mybir.AluOpType.subtract,
                                    mybir.AluOpType.min)
            st["t2"] = nt_sb

        def emit_smat_w(st):
            """smat select (PE) + w product (DVE)."""
            t2_sb = st["t2"]
            t2y_ps = psB.tile([32, T], F32, tag="small")
            mm(t2y_ps, smat_s, t2_sb, start=True, stop=True)
            w_sb = work.tile([32, T], F32, tag="w")
            nc.vector.tensor_mul(w_sb, t2_sb[0:32, :], t2y_ps)
            st["w"] = w_sb

        def emit_lg(st):
            lg_sb = work.tile([32, T], F32, tag="lg")
            nc.vector.tensor_mul(lg_sb, st["qk"], st["w"])
            st["lg"] = lg_sb

        def emit_e(st):
            e_sb = work.tile([32, T], F32R, tag="e")
            nc.scalar.activation(e_sb, st["lg"], AF.Exp, bias=0.0, scale=SIGMA)
            st["e"] = e_sb

        def emit_ew(st):
            ew_sb = work.tile([32, T], F32R, tag="ew")
            nc.gpsimd.tensor_mul(ew_sb, st["e"], st["w"])
            st["ew"] = ew_sb

        def emit_ssum(st):
            s1_ps = psB.tile([8, T], F32, tag="small")
            mm(s1_ps, cmat_s, st["e"], start=True, stop=True)
            s2_ps = psB.tile([8, T], F32, tag="small")
            mm(s2_ps, cmat_s, st["ew"], start=True, stop=True)
            st["s1"], st["s2"] = s1_ps, s2_ps

        def emit_norm(st):
            r1_sb = work.tile([8, T], F32, tag="r1")
            nc.vector.reciprocal(r1_sb, st["s1"])
            wv_sb = work.tile([8, T], F32R, tag="wvv")
            nc.vector.tensor_mul(wv_sb, st["s2"], r1_sb)
            st["wv"] = wv_sb

        def emit_ov(st):
            ov_sb = work.tile([128, 2, T], F32R, tag="ov")
            st["ov"] = ov_sb
            for mc in range(2):
                wvx_ps = psB.tile([128, T], F32, tag="small")
                mm(wvx_ps, bmat_s[:, mc * 128:(mc + 1) * 128], st["wv"],
                   start=True, stop=True)
                nc.vector.tensor_mul(ov_sb[:, mc, :], st["v"][:, mc, :], wvx_ps)

        def emit_outproj_mm(st):
            o_sb = work.tile([128, 2, T], BF16, tag="osb")
            st["osb"] = o_sb
            st["ops"] = []
            for mc in range(2):
                o_ps = psB.tile([128, T], F32, tag="small")
                m128 = slice(mc * 128, (mc + 1) * 128)
                mm(o_ps, wo_s[:, 0, m128], st["ov"][:, 0, :], start=True,
                   stop=False)
                mm(o_ps, wo_s[:, 1, m128], st["ov"][:, 1, :], start=False,
                   stop=not with_bias)
                if with_bias:
                    mm(o_ps, bor_s[:, m128], ones_s[:, :T], start=False,
                       stop=True)
                st["ops"].append(o_ps)

        def emit_ocopies(st):
            g0 = st["g0"]
            nc.scalar.copy(st["osb"][:, 0, :], st["ops"][0])
            nc.sync.dma_start(out=out[:, 0, g0:g0 + T], in_=st["osb"][:, 0, :])
            nc.vector.tensor_copy(st["osb"][:, 1, :], st["ops"][1])
            nc.sync.dma_start(out=out[:, 1, g0:g0 + T], in_=st["osb"][:, 1, :])

        # ---------- startup loads fanned out over four DGE queues (SP, ACT,
        # DVE, Pool) so the first iteration's inputs arrive in parallel ----
        wp_s = singles.tile([128, 2, 256], F32R, name="sb_wp")
        nc.sync.dma_start(out=wp_s, in_=dram["wp"][:])
        wq_s = singles.tile([128, 2, 256], F32R, name="sb_wq")
        nc.scalar.dma_start(out=wq_s, in_=dram["wq"][:])
        wk_s = singles.tile([128, 2, 256], F32R, name="sb_wk")
        nc.gpsimd.dma_start(out=wk_s, in_=dram["wk"][:])
        wv_s = singles.tile([128, 2, 256], F32R, name="sb_wv")
        nc.gpsimd.dma_start(out=wv_s, in_=dram["wv"][:])
        xp_0 = inp.tile([128, 2, tload], F32R, tag="xp")
        nc.sync.dma_start(out=xp_0, in_=xp[:, :, 0:tload])
        xq_0 = inp.tile([128, 2, tload], F32R, tag="xq")
        nc.scalar.dma_start(out=xq_0, in_=xq[:, :, 0:tload])
        xk_0 = inp.tile([128, 2, tload], F32R, tag="xk")
        nc.gpsimd.dma_start(out=xk_0, in_=xk[:, :, 0:tload])
        xv_0 = inp.tile([128, 2, tload], F32R, tag="xv")
        nc.gpsimd.dma_start(out=xv_0, in_=xv[:, :, 0:tload])
        ref_0 = inp.tile([2, tload], F32R, tag="ref")
        nc.gpsimd.dma_start(out=ref_0, in_=ref[:, 0:tload])
        ld0 = (xq_0, xp_0, xk_0, xv_0, ref_0)
        wo1_s = singles.tile([128, 2, 512], F32R, name="sb_wo1")
        nc.scalar.dma_start(out=wo1_s, in_=dram["wo1"][:])
        bo1_s = load1("bo1", (128, 4))
        amat_s = load1("amat", (128, 64))
        bwof_s = load1("bwof", (64, 1))
        smat_s = load1("smat", (64, 32))
        cmat_s = load1("cmat", (32, 8))
        wo2_s = load1("wo2", (128, 4, 64))
        pmat_s = load1("pmat", (2, 64))
        bmat_s = load1("bmat", (8, 256))
        wo_s = load1("wo", (128, 2, 256))
        if with_bias:
            bqr_s = load1("bqr", (1, 256))
            bkr_s = load1("bkr", (1, 256))
            bpr_s = load1("bpr", (1, 256))
            bvr_s = load1("bvr", (1, 256))
            bor_s = load1("bor", (1, 256))
            ones_s = load1("ones", (1, 512))

        # ---------- skewed pipeline ----------
        # iteration i emits: stage1(i) | t1/t2(i-1) | softmax+ov chain(i-2)
        # | out-projection + store(i-3).  The chain's inputs (t2, qk, m) are
        # then a full iteration old when the PE/DVE/ACT reach them, so no
        # engine waits on a same-iteration producer.
        states = {}
        for it in range(nload + 3):
            cur = states.get(it)
            prev = states.get(it - 1)
            prv2 = states.get(it - 2)
            prv3 = states.get(it - 3)
            if it < nload:
                cur = states[it] = {
                    "g0": it * tload,
                    "ld": ld0 if it == 0 else states[it - 1]["ld_next"],
                    "v": work.tile([128, 2, T], F32, tag="v", bufs=4,
                                   name="v_sb"),
                    "pp": work.tile([128, 2, T], F32, tag="pp", bufs=2,
                                    name="pp_sb"),
                    "tq": work.tile([128, 2, T], F32, tag="tq", bufs=2,
                                    name="tq_sb"),
                    "tk": work.tile([128, 2, T], F32, tag="tk", bufs=2,
                                    name="tk_sb"),
                    "m": work.tile([128, 2, T], F32R, tag="m", bufs=3,
                                   name="m_sb"),
                }
            # per-engine queue orders (steady state):
            #  ACT:  t1(p1), pp0, v0, relu x4, pp1, v1, e(p1), oc0(p3)
            #  DVE:  recip(p2), wv(p2), tq0, tk0, w(p1), lg(p1), tq1, tk1,
            #        ov0(p2), ov1(p2), oc1(p3)
            #  Pool: ew(p2), nt(p1), m0, m1
            #  PE:   pp0,q0 | s1,s2(p2) | k0,v0 | hid x8 | qk(p1) | smat(p1)
            #        | pp1,q1 | k1,v1 | off | wvx(p2) | outproj(p3)
            if prev is not None:
                emit_t1(prev)            # ACT: t1
            if prv2 is not None:
                emit_ew(prv2)            # Pool: ew
            if cur is not None:
                emit_chunk_pq(cur, 0)    # PE: pp0,q0  ACT: pp0-copy
            if prv2 is not None:
                emit_ssum(prv2)          # PE: s1, s2
                emit_norm(prv2)          # DVE: recip, wv
            if prev is not None:
                emit_nt(prev)            # Pool: nt
            if cur is not None:
                if it == 0:
                    emit_hid(cur)
                emit_chunk_k(cur, 0)     # PE: k0
                emit_chunk_v(cur, 0)     # PE: v0      ACT: v0-copy
                emit_adds(cur, 0)        # DVE: tq0, tk0
                if it + 1 < nload:
                    cur["ld_next"] = load_tile(it + 1)
                if it > 0:
                    emit_hid(cur)        # PE: hid x8  ACT: relu x4
            if prev is not None:
                emit_qk(prev)            # PE: qk amat
                emit_smat_w(prev)        # PE: smat    DVE: w
                emit_lg(prev)            # DVE: lg
            if cur is not None:
                emit_chunk_pq(cur, 1)    # PE: pp1,q1  ACT: pp1-copy
                emit_chunk_k(cur, 1)     # PE: k1
                emit_chunk_v(cur, 1)     # PE: v1      ACT: v1-copy
            if prev is not None:
                emit_e(prev)             # ACT: e
            if cur is not None:
                emit_adds(cur, 1)        # DVE: tq1, tk1
                emit_mmuls(cur)          # Pool: m0, m1
                emit_off(cur)            # PE: off x5
            if prv2 is not None:
                emit_ov(prv2)            # PE: wvx x2  DVE: ov0, ov1
            if prv3 is not None:
                emit_outproj_mm(prv3)    # PE: outproj x4
                emit_ocopies(prv3)       # ACT: oc0  DVE: oc1  + DMA out

    nc.compile()
    return nc


def _consts():
    amat = np.zeros((128, 64), np.float32)
    for mc in range(2):
        for d in range(128):
            h = mc * 4 + d // 32
            for k in range(KP):
                amat[d, mc * 32 + h * KP + k] = 1.0
    cmat = np.zeros((32, 8), np.float32)
    for j in range(32):
        cmat[j, j // KP] = 1.0
    bmat = np.zeros((8, 256), np.float32)
    for mc in range(2):
        for c in range(128):
            bmat[mc * 4 + c // 32, mc * 128 + c] = 1.0
    pmat = np.zeros((2, 64), np.float32)
    for r in range(64):
        pmat[r // 32, r] = 1.0
    smat = np.zeros((64, 32), np.float32)
    for j in range(32):
        smat[32 + j, j] = 1.0
    return amat, cmat, bmat, pmat, smat


def _wsplit(w):
    # [256, O] -> [128, 2, O]  (row kc*128+p  ->  [p, kc, :])
    o = w.shape[1]
    return np.ascontiguousarray(w.reshape(2, 128, o).transpose(1, 0, 2))


def _xsplit(x):
    # [T, 256] token-major -> [128, 2, T] channel-major chunks
    t = x.shape[0]
    return np.ascontiguousarray(x.T.reshape(2, 128, t).transpose(1, 0, 2))


def _host_maps(inputs, toks, ncores):
    f32 = lambda v: np.asarray(v, dtype=np.float32)
    query = f32(inputs["query"]).reshape(-1, C)
    key = f32(inputs["key"]).reshape(-1, C)
    value = f32(inputs["value"]).reshape(-1, C)
    pos = f32(inputs["pos_embed"]).reshape(-1, C)
    refp = f32(inputs["reference_points"]).reshape(-1, 2)

    # permute Wo2 columns (h,k,c) -> (c,h,k)
    perm = [h * (KP * 2) + k * 2 + c for c in range(2) for h in range(H)
            for k in range(KP)]
    wo2p = f32(inputs["Wo2"])[:, perm]
    bo2p = f32(inputs["bo2"])[perm]

    amat, cmat, bmat, pmat, smat = _consts()
    bq = f32(inputs["bq"])
    bk = f32(inputs["bk"])
    bp = f32(inputs["bpos"])
    bv = f32(inputs["bv"])
    bout = f32(inputs["bout"])
    with_bias = any(np.any(b != 0) for b in (bq, bk, bp, bv, bout))

    wo2r = np.ascontiguousarray(wo2p.reshape(4, 128, 64).transpose(1, 0, 2))
    shared = {
        "wq": _wsplit(f32(inputs["Wq"])),
        "wk": _wsplit(f32(inputs["Wk"])),
        "wv": _wsplit(f32(inputs["Wv"])),
        "wp": _wsplit(f32(inputs["Wpos"])),
        "wo1": _wsplit(f32(inputs["Wo1"])),
        "wo2": wo2r,
        "wo": _wsplit(f32(inputs["Wout"])),
        "bo1": np.ascontiguousarray(f32(inputs["bo1"]).reshape(4, 128).T),
        "bwof": np.ascontiguousarray((bo2p - 0.5).reshape(64, 1)),
        "smat": smat,
        "amat": amat, "cmat": cmat, "bmat": bmat, "pmat": pmat,
    }
    if with_bias:
        shared["ones"] = np.ones((1, 512), np.float32)
        shared["bqr"] = bq.reshape(1, 256)
        shared["bkr"] = bk.reshape(1, 256)
        shared["bpr"] = bp.reshape(1, 256)
        shared["bvr"] = bv.reshape(1, 256)
        shared["bor"] = bout.reshape(1, 256)

    in_maps = []
    for cid in range(ncores):
        sl = slice(cid * toks, (cid + 1) * toks)
        m = dict(shared)
        m["xq"] = _xsplit(query[sl])
        m["xk"] = _xsplit(key[sl])
        m["xv"] = _xsplit(value[sl])
        m["xp"] = _xsplit(pos[sl])
        m["ref"] = np.ascontiguousarray(refp[sl].T)
        in_maps.append(m)
    return in_maps, with_bias


_NC_CACHE = {}


def kernel(**inputs):
    from concourse.bass_utils import run_bass_kernel_spmd

    in_maps, with_bias = _host_maps(inputs, TOKS, NCORES)
    ck = ("full", with_bias)
    if ck not in _NC_CACHE:
        _NC_CACHE[ck] = _build(toks=TOKS, tload=TLOAD, with_bias=with_bias)
    nc = _NC_CACHE[ck]
    res = run_bass_kernel_spmd(nc, in_maps, core_ids=list(range(NCORES)))
    outs = []
    for r in res.results:
        # [128, 2, toks] channel-major -> [toks, 256] token-major
        o = np.asarray(r["out"])
        outs.append(o.transpose(2, 1, 0).reshape(TOKS, 256))
    full = np.concatenate(outs, axis=0).reshape(N, L, C)
    return np.ascontiguousarray(full.astype(np.float32))
